# revision 5
# baseline (speedup 1.0000x reference)
"""2D Haar DWT (level 1) Trainium2 Bass kernel — fp16 pipeline, DVE+GpSimd.

Input  x: [16, 64, 256, 256] f32
Output y: [16, 256, 128, 128] f32, y[n, s*64+c, i, j] = Haar mix s of the
2x2 block x[n, c, 2i:2i+2, 2j:2j+2].

Sharding: pure data parallel over the batch dim — core k gets batches
[2k, 2k+2).

All device traffic is fp16 (tolerance is 2e-2 relative, fp16 keeps us
~8e-4): the host converts x f32->fp16 before upload and converts the fp16
result back to f32, folding the Haar 0.5 normalization into the exact
host-side scale. Per-core HBM traffic is 16+16 MiB -> ~94 us roofline.

Per-core design, per group of G=16 channel planes (2 MiB contiguous DRAM):
  load:    pure reshape -> it[p=(c*8+q), o=32 rows, w=256], 16 KB/partition
  stage 1 (vertical):   sd[v=0] = rows 2t + rows 2t+1, sd[v=1] = diff
           (step-1 fp16 -> DVE 2x mode, ~2.2 us/op)
  stage 2 (horizontal): even/odd column butterfly -> oadd (subbands 0,1),
           osub (subbands 2,3). Stride-2 fp16 reads run at DVE 1x, so the
           row-pair range is split: DVE takes r < R_DVE, GpSimd (idle
           otherwise, ~2.6 cyc/elem) takes r >= R_DVE.
  stores:  groups are processed in pairs writing one double-width out tile
           so each store is [128, 4096] = 1 MiB contiguous, 4 KB runs.

All DMAs go through the two HWDGE rings (sync + scalar queues).
"""

import sys

sys.path.insert(0, "/opt/trn_rl_repo")

import numpy as np

import concourse.bacc as bacc
import concourse.mybir as mybir
from concourse.tile import TileContext

N_CORES = 8
N_PER_CORE = 2  # batches per core
C = 64  # input channels
H = 256
W = 256
G = 16  # channels per group (2 MB loads, 32 rows/partition)
R_DVE = 11  # stage-2 row-pair split: DVE r<R_DVE, GpSimd r>=R_DVE
F16 = mybir.dt.float16


def build_nc():
    nc = bacc.Bacc("TRN2", target_bir_lowering=False, debug=False)
    x = nc.dram_tensor("x", [N_PER_CORE, C, H, W], F16, kind="ExternalInput")
    y = nc.dram_tensor("y", [N_PER_CORE, 4 * C, H // 2, W // 2], F16, kind="ExternalOutput")

    with TileContext(nc) as tc:
        with (
            tc.tile_pool(name="inpool", bufs=4) as inpool,
            tc.tile_pool(name="sdpool", bufs=3) as sdpool,
            tc.tile_pool(name="outpool", bufs=2) as outpool,
        ):
            pi = 0
            for n in range(N_PER_CORE):
                for c00 in range(0, C, 2 * G):  # pair of groups -> one store set
                    oadd = outpool.tile([128, 2 * G * 256], F16, tag="oadd")
                    osub = outpool.tile([128, 2 * G * 256], F16, tag="osub")
                    oadd_v = oadd[:].rearrange("p (v g r j) -> p v g r j", v=2, g=2, r=G)
                    osub_v = osub[:].rearrange("p (v g r j) -> p v g r j", v=2, g=2, r=G)

                    for g in range(2):
                        c0 = c00 + g * G
                        # --- load: pure reshape of the 2 MB contiguous group.
                        # it[p, o, w] = x[n, c0 + p//8, 32*(p%8) + o, w]
                        it = inpool.tile([128, G * 512], F16, tag="in")
                        src = x[n, c0 : c0 + G].rearrange(
                            "c (q o) w -> (c q) o w", o=2 * G
                        )
                        nc.sync.dma_start(
                            out=it[:].rearrange("p (o w) -> p o w", o=2 * G), in_=src
                        )

                        # --- stage 1 (vertical): rows 2t / 2t+1 within a partition
                        itv = it[:].rearrange("p (r t w) -> p r t w", r=G, t=2)
                        sd = sdpool.tile([128, G * 512], F16, tag="sd")
                        sdv = sd[:].rearrange("p (v r w) -> p v r w", v=2, r=G)
                        nc.vector.tensor_add(
                            out=sdv[:, 0], in0=itv[:, :, 0, :], in1=itv[:, :, 1, :]
                        )
                        nc.vector.tensor_sub(
                            out=sdv[:, 1], in0=itv[:, :, 0, :], in1=itv[:, :, 1, :]
                        )

                        # --- stage 2 (horizontal): even/odd column butterfly,
                        # r-range split across DVE and GpSimd.
                        # (the Haar 0.5 normalization is applied host-side)
                        sdj = sd[:].rearrange(
                            "p (v r j t) -> p v r j t", v=2, r=G, t=2
                        )
                        nc.vector.tensor_add(
                            out=oadd_v[:, :, g, :R_DVE],
                            in0=sdj[:, :, :R_DVE, :, 0],
                            in1=sdj[:, :, :R_DVE, :, 1],
                        )
                        nc.vector.tensor_sub(
                            out=osub_v[:, :, g, :R_DVE],
                            in0=sdj[:, :, :R_DVE, :, 0],
                            in1=sdj[:, :, :R_DVE, :, 1],
                        )
                        nc.gpsimd.tensor_add(
                            out=oadd_v[:, :, g, R_DVE:],
                            in0=sdj[:, :, R_DVE:, :, 0],
                            in1=sdj[:, :, R_DVE:, :, 1],
                        )
                        nc.gpsimd.tensor_sub(
                            out=osub_v[:, :, g, R_DVE:],
                            in0=sdj[:, :, R_DVE:, :, 0],
                            in1=sdj[:, :, R_DVE:, :, 1],
                        )

                    # --- stores: (tile, v, subband) -> 1 MiB contiguous each;
                    # output row i = 16*(p%8) + r, channel c0 + g*G + p//8.
                    for t_, v, s in ((oadd, 0, 0), (oadd, 1, 1), (osub, 0, 2), (osub, 1, 3)):
                        dst = y[n, s * C + c00 : s * C + c00 + 2 * G].rearrange(
                            "(g c) (q r) j -> (c q) g r j", g=2, r=G
                        )
                        eng = nc.sync if (pi * 4 + s) % 2 == 0 else nc.scalar
                        eng.dma_start(
                            out=dst,
                            in_=t_[:].rearrange(
                                "p (v g r j) -> p v g r j", v=2, g=2, r=G
                            )[:, v],
                        )
                    pi += 1

    nc.finalize()
    return nc


_NC = None


def _get_nc():
    global _NC
    if _NC is None:
        _NC = build_nc()
    return _NC


def prep_in_maps(x: np.ndarray) -> list:
    """f32 full input -> per-core fp16 input maps."""
    x16 = np.ascontiguousarray(x, dtype=np.float16)
    return [
        {"x": x16[k * N_PER_CORE : (k + 1) * N_PER_CORE]} for k in range(N_CORES)
    ]


def post_results(results: list) -> np.ndarray:
    """Per-core fp16 outputs -> full f32 output (applies the Haar 0.5)."""
    y16 = np.concatenate([r["y"] for r in results], axis=0)
    return y16.astype(np.float32) * np.float32(0.5)


def kernel(x: np.ndarray) -> np.ndarray:
    from concourse.bass_utils import run_bass_kernel_spmd

    x = np.asarray(x)
    assert x.shape == (16, C, H, W), x.shape

    nc = _get_nc()
    res = run_bass_kernel_spmd(nc, prep_in_maps(x), core_ids=list(range(N_CORES)))
    return post_results(res.results)


# revision 6
# speedup vs baseline: 1.1472x; 1.1472x over previous
"""2D Haar DWT (level 1) Trainium2 Bass kernel — fp16 pipeline, all-DVE.

Input  x: [16, 64, 256, 256] f32
Output y: [16, 256, 128, 128] f32, y[n, s*64+c, i, j] = Haar mix s of the
2x2 block x[n, c, 2i:2i+2, 2j:2j+2].

Sharding: pure data parallel over the batch dim — core k gets batches
[2k, 2k+2).

All device traffic is fp16 (tolerance is 2e-2 relative, fp16 keeps us
~8e-4): the host converts x f32->fp16 before upload and converts the fp16
result back to f32, folding the Haar 0.5 normalization into the exact
host-side scale. Per-core HBM traffic is 16+16 MiB -> ~94 us roofline;
the DVE butterfly floor (1.5 cyc/elem: vertical stage in 2x mode,
horizontal stage stride-2 at 1x) is ~105 us and is the binding constraint.
GpSimd offload of the 1x stage was measured and REGRESSES (the shared
second SBUF port inflates DVE tensor_tensor ~40% while GpSimd runs).

Per-core design, per group of G=16 channel planes (2 MiB contiguous DRAM):
  load:    pure reshape -> it[p=(c*8+q), o=32 rows, w=256], 16 KB/partition.
           The very first group's load is split into 4 x 512 KB chunks on
           alternating HWDGE queues so DVE starts ~8 us earlier.
  stage 1 (vertical):   sd[v=0] = rows 2t + rows 2t+1, sd[v=1] = diff
           (step-1 fp16 -> DVE 2x mode, ~2.2 us/op)
  stage 2 (horizontal): even/odd column butterfly -> oadd (subbands 0,1),
           osub (subbands 2,3); stride-2 reads -> DVE 1x (~4.3 us/op)
  stores:  groups are processed in pairs writing one double-width out tile
           so each store is [128, 4096] = 1 MiB contiguous (4 KB runs);
           the final pair stays unpaired (4+4 x 512 KB) to shorten the tail.

All DMAs go through the two HWDGE rings (sync + scalar queues).
"""

import sys

sys.path.insert(0, "/opt/trn_rl_repo")

import numpy as np

import concourse.bacc as bacc
import concourse.mybir as mybir
from concourse.tile import TileContext

N_CORES = 8
N_PER_CORE = 2  # batches per core
C = 64  # input channels
H = 256
W = 256
G = 16  # channels per group (2 MB loads, 32 rows/partition)
F16 = mybir.dt.float16


def build_nc():
    nc = bacc.Bacc("TRN2", target_bir_lowering=False, debug=False)
    x = nc.dram_tensor("x", [N_PER_CORE, C, H, W], F16, kind="ExternalInput")
    y = nc.dram_tensor("y", [N_PER_CORE, 4 * C, H // 2, W // 2], F16, kind="ExternalOutput")

    n_pairs = N_PER_CORE * C // (2 * G)  # pairs of groups sharing a store set

    with TileContext(nc) as tc:
        with (
            tc.tile_pool(name="inpool", bufs=4) as inpool,
            tc.tile_pool(name="sdpool", bufs=3) as sdpool,
            tc.tile_pool(name="outpool", bufs=2) as outpool,
        ):
            pi = 0
            for n in range(N_PER_CORE):
                for c00 in range(0, C, 2 * G):
                    last_pair = pi == n_pairs - 1
                    oadd = outpool.tile([128, 2 * G * 256], F16, tag="oadd")
                    osub = outpool.tile([128, 2 * G * 256], F16, tag="osub")
                    oadd_v = oadd[:].rearrange("p (v g r j) -> p v g r j", v=2, g=2, r=G)
                    osub_v = osub[:].rearrange("p (v g r j) -> p v g r j", v=2, g=2, r=G)

                    for g in range(2):
                        c0 = c00 + g * G
                        # --- load: pure reshape of the 2 MB contiguous group.
                        # it[p, o, w] = x[n, c0 + p//8, 32*(p%8) + o, w]
                        it = inpool.tile([128, G * 512], F16, tag="in")
                        itd = it[:].rearrange("p (o w) -> p o w", o=2 * G)
                        src = x[n, c0 : c0 + G].rearrange(
                            "c (q o) w -> (c q) o w", o=2 * G
                        )
                        if pi == 0 and g == 0:
                            # split first load into 4 x 512 KB on both queues
                            for k in range(4):
                                eng = nc.sync if k % 2 == 0 else nc.scalar
                                eng.dma_start(
                                    out=itd[:, 8 * k : 8 * k + 8],
                                    in_=src[:, 8 * k : 8 * k + 8],
                                )
                        else:
                            nc.sync.dma_start(out=itd, in_=src)

                        # --- stage 1 (vertical): rows 2t / 2t+1 in a partition
                        itv = it[:].rearrange("p (r t w) -> p r t w", r=G, t=2)
                        sd = sdpool.tile([128, G * 512], F16, tag="sd")
                        sdv = sd[:].rearrange("p (v r w) -> p v r w", v=2, r=G)
                        if pi == 0 and g == 0:
                            # chunked to start right after the first 512 KB
                            for k in range(4):
                                rs = slice(4 * k, 4 * k + 4)
                                nc.vector.tensor_add(
                                    out=sdv[:, 0, rs],
                                    in0=itv[:, rs, 0, :],
                                    in1=itv[:, rs, 1, :],
                                )
                                nc.vector.tensor_sub(
                                    out=sdv[:, 1, rs],
                                    in0=itv[:, rs, 0, :],
                                    in1=itv[:, rs, 1, :],
                                )
                        else:
                            nc.vector.tensor_add(
                                out=sdv[:, 0], in0=itv[:, :, 0, :], in1=itv[:, :, 1, :]
                            )
                            nc.vector.tensor_sub(
                                out=sdv[:, 1], in0=itv[:, :, 0, :], in1=itv[:, :, 1, :]
                            )

                        # --- stage 2 (horizontal): even/odd column butterfly.
                        # (the Haar 0.5 normalization is applied host-side)
                        sdj = sd[:].rearrange(
                            "p (v r j t) -> p v r j t", v=2, r=G, t=2
                        )
                        nc.vector.tensor_add(
                            out=oadd_v[:, :, g],
                            in0=sdj[..., 0],
                            in1=sdj[..., 1],
                        )
                        nc.vector.tensor_sub(
                            out=osub_v[:, :, g],
                            in0=sdj[..., 0],
                            in1=sdj[..., 1],
                        )

                        # --- unpaired 512 KB stores for the last pair: each
                        # group's outputs leave as soon as they are ready.
                        if last_pair:
                            for t_, v, s in (
                                (oadd, 0, 0), (oadd, 1, 1), (osub, 0, 2), (osub, 1, 3),
                            ):
                                dst = y[n, s * C + c0 : s * C + c0 + G].rearrange(
                                    "c (q r) j -> (c q) r j", r=G
                                )
                                eng = nc.sync if s % 2 == 0 else nc.scalar
                                eng.dma_start(
                                    out=dst,
                                    in_=t_[:].rearrange(
                                        "p (v g r j) -> p v g r j", v=2, g=2, r=G
                                    )[:, v, g],
                                )

                    # --- paired stores: 1 MiB contiguous each;
                    # output row i = 16*(p%8) + r, channel c00 + g*G + p//8.
                    if not last_pair:
                        for t_, v, s in (
                            (oadd, 0, 0), (oadd, 1, 1), (osub, 0, 2), (osub, 1, 3),
                        ):
                            dst = y[n, s * C + c00 : s * C + c00 + 2 * G].rearrange(
                                "(g c) (q r) j -> (c q) g r j", g=2, r=G
                            )
                            eng = nc.sync if (pi * 4 + s) % 2 == 0 else nc.scalar
                            eng.dma_start(
                                out=dst,
                                in_=t_[:].rearrange(
                                    "p (v g r j) -> p v g r j", v=2, g=2, r=G
                                )[:, v],
                            )
                    pi += 1

    nc.finalize()
    return nc


_NC = None


def _get_nc():
    global _NC
    if _NC is None:
        _NC = build_nc()
    return _NC


def prep_in_maps(x: np.ndarray) -> list:
    """f32 full input -> per-core fp16 input maps."""
    x16 = np.ascontiguousarray(x, dtype=np.float16)
    return [
        {"x": x16[k * N_PER_CORE : (k + 1) * N_PER_CORE]} for k in range(N_CORES)
    ]


def post_results(results: list) -> np.ndarray:
    """Per-core fp16 outputs -> full f32 output (applies the Haar 0.5)."""
    y16 = np.concatenate([r["y"] for r in results], axis=0)
    return y16.astype(np.float32) * np.float32(0.5)


def kernel(x: np.ndarray) -> np.ndarray:
    from concourse.bass_utils import run_bass_kernel_spmd

    x = np.asarray(x)
    assert x.shape == (16, C, H, W), x.shape

    nc = _get_nc()
    res = run_bass_kernel_spmd(nc, prep_in_maps(x), core_ids=list(range(N_CORES)))
    return post_results(res.results)
